# revision 3
# baseline (speedup 1.0000x reference)
"""Chamfer loss (chunked) Trainium2 kernel — nn_ChamferLoss_8194797601432.

Reference math: for each 2048-point chunk c of pc1, compute the vector
min over the chunk of ||pc2[m] - pc1_chunk[p]||^2 for all m in pc2 (and
symmetrically for chunks of pc2 vs pc1), concatenate, return
mean(dist1) + mean(dist2)  (scalar fp32).

Device strategy (8 NeuronCores, SPMD, per-core data):
  Core c handles chunk c for both halves (2 tasks per core):
    task := (ref = full opposite cloud [16384 pts], chunk = 2048 pts)
    G[m,p] = full squared distance, via K=24 bf16 matmuls per
    (m-tile of 128 ref pts, p-slice of 512 chunk pts):
      - 3-term bf16 split of every coordinate (Ozaki-style):
        a ~ a0+a1+a2, b ~ b0+b1+b2; 6 product pairs (s,t) with s+t<=2
        per dim = 18 rows; + 3 rows ||b_p||^2 split (ref side ones);
        + 3 rows ||a_m||^2 split (chunk side ones).  bf16 products are
        exact in the PE and accumulate in fp32 PSUM: elementwise error
        ~1e-5 abs (HW-verified exact vs fp32 emulation on probes),
        vs ~3e-4 for fp32r which failed the rel tolerance.
      - PE cost is N_moving cycles/row independent of K, so K=24 is free.
    min over p (2048) per m on VectorE tensor_tensor_scan(min, min),
    in0 = PSUM, in1 = ScalarE PSUM->SBUF copy (tensor_tensor_reduce
    would fuse the final column extraction but crashes real TRN2 HW).
    Final-column extraction copies alternate ScalarE/VectorE by m-tile
    parity to balance engine load.
  Host: concatenate, mean in float64, cast fp32.
"""

import numpy as np

NPTS = 16384
NCHUNK = 2048
NCORES = 8
NM = NPTS // 128  # 128 m-tiles per task
NTASKS = 2
K = 24

# "scan1024": one FD=1024 scan per m-tile (in0 spans 2 PSUM banks).
# "scan512": two chained FD=512 scans per m-tile (single-bank reads).
VARIANT = "scan512"

_CACHE = {}


def _build(reps=1, variant=VARIANT):
    import concourse.bacc as bacc
    import concourse.mybir as mybir
    import concourse.tile as tile
    from contextlib import ExitStack

    FP32 = mybir.dt.float32
    BF16 = mybir.dt.bfloat16
    MIN = mybir.AluOpType.min

    nc = bacc.Bacc("TRN2", target_bir_lowering=False)

    refs = [
        nc.dram_tensor(f"ref{t}", [K, NPTS], BF16, kind="ExternalInput")
        for t in range(NTASKS)
    ]
    chunks = [
        nc.dram_tensor(f"chunk{t}", [K, NCHUNK], BF16, kind="ExternalInput")
        for t in range(NTASKS)
    ]
    minout = nc.dram_tensor("minout", [NTASKS, 128, NM], FP32, kind="ExternalOutput")

    with tile.TileContext(nc) as tc:
        with ExitStack() as ctx:
            const_pool = ctx.enter_context(tc.tile_pool(name="const", bufs=1))
            psum_pool = ctx.enter_context(
                tc.tile_pool(
                    name="psum", bufs=4 if variant == "scan1024" else 8, space="PSUM"
                )
            )
            scp_pool = ctx.enter_context(tc.tile_pool(name="scp", bufs=3))
            scr_pool = ctx.enter_context(tc.tile_pool(name="scr", bufs=3))
            out_pool = ctx.enter_context(tc.tile_pool(name="out", bufs=1))

            ref_ts, chunk_ts, minbufs = [], [], []
            for t in range(NTASKS):
                ref_t = const_pool.tile([K, NPTS], BF16, tag=f"ref{t}", name=f"ref_{t}")
                chunk_t = const_pool.tile(
                    [K, NCHUNK], BF16, tag=f"chunk{t}", name=f"chunk_{t}"
                )
                nc.sync.dma_start(ref_t[:], refs[t][:])
                nc.sync.dma_start(chunk_t[:], chunks[t][:])
                ref_ts.append(ref_t)
                chunk_ts.append(chunk_t)
                minbufs.append(
                    out_pool.tile([128, NM], FP32, tag=f"minbuf{t}", name=f"minbuf{t}")
                )

            loop_cm = tc.For_i(0, reps, 1) if reps > 1 else None
            if loop_cm is not None:
                loop_cm.__enter__()

            for t in range(NTASKS):
                ref_t, chunk_t, minbuf = ref_ts[t], chunk_ts[t], minbufs[t]
                for mt in range(NM):
                    lhsT = ref_t[:, mt * 128 : (mt + 1) * 128]
                    if variant == "scan1024":
                        pb = psum_pool.tile([128, 1024], FP32, tag="ps", name=f"pb_{t}_{mt}")
                        pa = psum_pool.tile([128, 1024], FP32, tag="ps", name=f"pa_{t}_{mt}")
                        for j, dst in enumerate(
                            (pb[:, 0:512], pb[:, 512:1024], pa[:, 0:512], pa[:, 512:1024])
                        ):
                            nc.tensor.matmul(
                                dst,
                                lhsT=lhsT,
                                rhs=chunk_t[:, j * 512 : (j + 1) * 512],
                                start=True,
                                stop=True,
                            )
                        sb = scp_pool.tile([128, 1024], FP32, tag="scp", name=f"sb_{t}_{mt}")
                        nc.scalar.copy(sb[:], pb[:])
                        scr = scr_pool.tile([128, 1024], FP32, tag="scr", name=f"scr_{t}_{mt}")
                        nc.vector.tensor_tensor_scan(
                            scr[:], pa[:], sb[:], initial=1e30, op0=MIN, op1=MIN
                        )
                        last = scr[:, 1023:1024]
                    else:
                        ps = [
                            psum_pool.tile([128, 512], FP32, tag="ps", name=f"ps_{t}_{mt}_{j}")
                            for j in range(4)
                        ]
                        # emit the Act-copied banks first so ScalarE starts early
                        for j in (1, 3, 0, 2):
                            nc.tensor.matmul(
                                ps[j][:],
                                lhsT=lhsT,
                                rhs=chunk_t[:, j * 512 : (j + 1) * 512],
                                start=True,
                                stop=True,
                            )
                        sbs = {}
                        for j in (1, 3):
                            sbs[j] = scp_pool.tile(
                                [128, 512], FP32, tag="scp", name=f"sb_{t}_{mt}_{j}"
                            )
                            nc.scalar.copy(sbs[j][:], ps[j][:])
                        scra = scr_pool.tile(
                            [128, 512], FP32, tag="scr", name=f"scra_{t}_{mt}"
                        )
                        nc.vector.tensor_tensor_scan(
                            scra[:], ps[0][:], sbs[1][:], initial=1e30, op0=MIN, op1=MIN
                        )
                        scrb = scr_pool.tile(
                            [128, 512], FP32, tag="scr", name=f"scrb_{t}_{mt}"
                        )
                        nc.vector.tensor_tensor_scan(
                            scrb[:],
                            ps[2][:],
                            sbs[3][:],
                            initial=scra[:, 511:512],
                            op0=MIN,
                            op1=MIN,
                        )
                        last = scrb[:, 511:512]
                    # alternate final-extraction engine to balance load
                    if mt % 2 == 0:
                        nc.scalar.copy(minbuf[:, mt : mt + 1], last)
                    else:
                        nc.vector.tensor_copy(minbuf[:, mt : mt + 1], last)

            if loop_cm is not None:
                loop_cm.__exit__(None, None, None)

            for t in range(NTASKS):
                nc.sync.dma_start(minout[t], minbufs[t][:])

    nc.compile()
    return nc


def get_nc(reps=1, variant=VARIANT):
    key = (reps, variant)
    if key not in _CACHE:
        _CACHE[key] = _build(reps, variant)
    return _CACHE[key]


def _split3(x):
    """3-term bf16 split: x ~ b0 + b1 + b2 (residual ~2^-27 |x|)."""
    import ml_dtypes

    bf = ml_dtypes.bfloat16
    x = x.astype(np.float32)
    b0 = x.astype(bf)
    r = x - b0.astype(np.float32)
    b1 = r.astype(bf)
    r2 = r - b1.astype(np.float32)
    b2 = r2.astype(bf)
    return [b0, b1, b2]


_P6 = [(0, 0), (0, 1), (1, 0), (0, 2), (2, 0), (1, 1)]


def _task_rows(ref_pts, chunk_pts):
    """Build [K, n_ref] and [K, n_chunk] bf16 row blocks for one task.

    G[m,p] = sum_k ref_rows[k][m] * chunk_rows[k][p] ~ ||a_m - b_p||^2
    """
    import ml_dtypes

    bf = ml_dtypes.bfloat16
    nR, nC = ref_pts.shape[0], chunk_pts.shape[0]
    aS = [_split3(ref_pts[:, d]) for d in range(3)]  # [dim][term][nR]
    bS = [_split3(chunk_pts[:, d]) for d in range(3)]
    ca = _split3((ref_pts.astype(np.float64) ** 2).sum(-1).astype(np.float32))
    cb = _split3((chunk_pts.astype(np.float64) ** 2).sum(-1).astype(np.float32))
    ones_R = np.ones(nR, bf)
    ones_C = np.ones(nC, bf)
    ref_rows, chunk_rows = [], []
    for d in range(3):
        for (s, t) in _P6:
            ref_rows.append(aS[d][s])
            # -2 * b_t is exact in bf16 (power-of-two scale)
            chunk_rows.append((-2.0 * bS[d][t].astype(np.float32)).astype(bf))
    for k in range(3):
        ref_rows.append(ones_R)
        chunk_rows.append(cb[k])
    for k in range(3):
        ref_rows.append(ca[k])
        chunk_rows.append(ones_C)
    assert len(ref_rows) == K and len(chunk_rows) == K
    return np.stack(ref_rows), np.stack(chunk_rows)


def _prep_in_maps(pc1, pc2):
    in_maps = []
    for c in range(NCORES):
        r0, c0 = _task_rows(pc2, pc1[c * NCHUNK : (c + 1) * NCHUNK])
        r1, c1 = _task_rows(pc1, pc2[c * NCHUNK : (c + 1) * NCHUNK])
        in_maps.append({"ref0": r0, "chunk0": c0, "ref1": r1, "chunk1": c1})
    return in_maps


def run_on_device(in_maps, reps=1, variant=VARIANT):
    from concourse.bass_utils import run_bass_kernel_spmd

    nc = get_nc(reps, variant)
    res = run_bass_kernel_spmd(nc, in_maps, core_ids=list(range(NCORES)))
    return res.results


def _postprocess(results, variant=VARIANT):
    d = np.empty((NTASKS, NCORES, NPTS), np.float64)
    for c in range(NCORES):
        mo = results[c]["minout"].astype(np.float64)  # [2, 128, NM]
        for t in range(NTASKS):
            d[t, c] = mo[t].T.reshape(-1)
    return np.array(d[0].mean() + d[1].mean(), dtype=np.float32)


def kernel(output_pc, gt_pc):
    pc1 = np.asarray(output_pc, dtype=np.float32).reshape(NPTS, 3)
    pc2 = np.asarray(gt_pc, dtype=np.float32).reshape(NPTS, 3)
    in_maps = _prep_in_maps(pc1, pc2)
    results = run_on_device(in_maps)
    return _postprocess(results)
